# revision 1
# baseline (speedup 1.0000x reference)
"""DeepFM forward on 8 Trainium2 NeuronCores (Bass/Tile).

Strategy: data-parallel batch split (2048 rows/core), embedding + first-order
tables combined into one [F*V, 17] f32 table replicated on every core (no
collectives).  Per 128-row tile: 26 SWDGE indirect row-gathers, FM first/second
order on DVE+ACT, PE transposes to feature-major; then a fp32 PE MLP with
biases, continuous linear term and the FM output folded in via host-prepared
weights.
"""
import sys
sys.path.insert(0, "/opt/trn_rl_repo")
import numpy as np
import jax
from jax.sharding import Mesh, PartitionSpec, NamedSharding
from jax.experimental.shard_map import shard_map

from concourse import bass, bacc, tile, mybir
from concourse.bass2jax import install_neuronx_cc_hook, _bass_exec_p, partition_id_tensor
from concourse.masks import make_identity

F32 = mybir.dt.float32
I32 = mybir.dt.int32
AF = mybir.ActivationFunctionType
ALU = mybir.AluOpType

F, V, E = 26, 100000, 16
CONT, H1, H2 = 13, 400, 400
B = 16384
NCORES = 8
BC = B // NCORES          # 2048 rows per core
NT = BC // 128            # 16 tiles of 128 rows
NBT = BC // 512           # 4 matmul b-tiles of 512
GW = 17                   # combined row width (16 emb + 1 first)
HD = 17                   # header cols: 13 cont, ones, fm, 2 pad
GF = HD + F * GW          # 459 total G columns
K1 = [(0, 128), (128, 256), (256, 384), (384, GF)]        # L1 K chunks
M1 = [(0, 128), (128, 256), (256, 384), (384, 401)]       # H chunks (L1 adds ones col)
M2 = [(0, 128), (128, 256), (256, 384), (384, 400)]       # L2 H chunks
K2 = [(0, 128), (128, 256), (256, 384), (384, 401)]       # L2 K chunks (400+ones)


def _build():
    nc = bacc.Bacc("TRN2", target_bir_lowering=False, debug=False,
                   num_devices=NCORES)
    gtab = nc.dram_tensor("gtab", [F * V, GW], F32, kind="ExternalInput").ap()
    idx_d = nc.dram_tensor("idx_d", [BC, F], I32, kind="ExternalInput").ap()
    cont_d = nc.dram_tensor("cont_d", [BC, CONT], F32, kind="ExternalInput").ap()
    w1_d = nc.dram_tensor("w1_d", [GF, H1 + 1], F32, kind="ExternalInput").ap()
    w2_d = nc.dram_tensor("w2_d", [401, H2], F32, kind="ExternalInput").ap()
    wot_d = nc.dram_tensor("wot_d", [128, 4], F32, kind="ExternalInput").ap()
    v0_d = nc.dram_tensor("v0_d", [128, 1], F32, kind="ExternalInput").ap()
    out_d = nc.dram_tensor("out_d", [1, BC], F32, kind="ExternalOutput").ap()

    with tile.TileContext(nc) as tc:
        with (
            tc.tile_pool(name="per", bufs=1) as per,
            tc.tile_pool(name="gp", bufs=3) as gp,
            tc.tile_pool(name="fm", bufs=2) as fmp,
            tc.tile_pool(name="pst", bufs=2, space="PSUM") as pst,
            tc.tile_pool(name="psm", bufs=2, space="PSUM") as psm,
            tc.tile_pool(name="psf", bufs=2, space="PSUM") as psf,
        ):
            # ---- persistent weights / outputs ----
            ident = per.tile([128, 128], F32, tag="ident")
            make_identity(nc, ident[:])
            w1c = []
            for ci, (a, b) in enumerate(K1):
                t = per.tile([b - a, H1 + 1], F32, tag=f"w1c{ci}", name=f"w1c{ci}")
                nc.sync.dma_start(t[:], w1_d[a:b, :])
                w1c.append(t)
            w2c = []
            for ci, (a, b) in enumerate(K2):
                t = per.tile([b - a, H2], F32, tag=f"w2c{ci}", name=f"w2c{ci}")
                nc.sync.dma_start(t[:], w2_d[a:b, :])
                w2c.append(t)
            wot = per.tile([128, 4], F32, tag="wot")
            nc.sync.dma_start(wot[:], wot_d[:])
            v0 = per.tile([128, 1], F32, tag="v0")
            nc.sync.dma_start(v0[:], v0_d[:])

            xt = []
            for ci, (a, b) in enumerate(K1):
                xt.append(per.tile([b - a, BC], F32, tag=f"xt{ci}", name=f"xt{ci}"))
            h1s = [per.tile([128, BC], F32, tag=f"h1s{c}", name=f"h1s{c}") for c in range(3)]
            h1s3 = per.tile([17, BC], F32, tag="h1s3")
            h2s = [per.tile([128, BC], F32, tag=f"h2s{c}", name=f"h2s{c}") for c in range(3)]
            h2s3 = per.tile([16, BC], F32, tag="h2s3")
            outsb = per.tile([1, BC], F32, tag="outsb")

            # ---- per-tile: gather + FM + transpose ----
            for t in range(NT):
                tsl = slice(t * 128, (t + 1) * 128)
                idxt = gp.tile([128, F], I32, tag="idxt")
                nc.sync.dma_start(idxt[:], idx_d[tsl, :])
                G = gp.tile([128, GF], F32, tag="g")
                nc.sync.dma_start(G[:, 0:CONT], cont_d[tsl, :])
                nc.vector.memset(G[:, 13:14], 1.0)   # ones col
                nc.vector.memset(G[:, 15:17], 0.0)   # pad cols
                for f in range(F):
                    nc.gpsimd.indirect_dma_start(
                        out=G[:, HD + GW * f: HD + GW * (f + 1)],
                        out_offset=None,
                        in_=gtab[:],
                        in_offset=bass.IndirectOffsetOnAxis(
                            ap=idxt[:, f:f + 1], axis=0),
                    )
                # FM: field-sum tree over [13|13] halves of the 26 groups
                s1 = fmp.tile([128, 221], F32, tag="s1")
                nc.vector.tensor_tensor(s1[:], G[:, 17:238], G[:, 238:459], op=ALU.add)
                s2 = fmp.tile([128, 102], F32, tag="s2")
                nc.vector.tensor_tensor(s2[:], s1[:, 0:102], s1[:, 102:204], op=ALU.add)
                s3 = fmp.tile([128, 51], F32, tag="s3")
                nc.vector.tensor_tensor(s3[:], s2[:, 0:51], s2[:, 51:102], op=ALU.add)
                s6 = fmp.tile([128, 17], F32, tag="s6")
                nc.vector.tensor_tensor(s6[:], s3[:, 0:17], s3[:, 17:34], op=ALU.add)
                nc.vector.tensor_tensor(s6[:], s6[:], s3[:, 34:51], op=ALU.add)
                nc.vector.tensor_tensor(s6[:], s6[:], s1[:, 204:221], op=ALU.add)
                # sum of squares over the whole gathered region (incl first col)
                sqs = fmp.tile([128, F * GW], F32, tag="sqs")
                sqall = fmp.tile([128, 1], F32, tag="sqall")
                nc.scalar.activation(sqs[:], G[:, HD:GF], AF.Square,
                                     accum_out=sqall[:])
                # minus sum of first^2 (strided 3D view over the first cols)
                gap = G[:]
                pstep = gap.ap[0][0]
                fview = bass.AP(gap.tensor, gap.offset + HD + 16,
                                [(pstep, 128), (GW, F), (1, 1)])
                sqf = fmp.tile([128, F], F32, tag="sqf")
                firstsq = fmp.tile([128, 1], F32, tag="firstsq")
                nc.scalar.activation(sqf[:], fview, AF.Square,
                                     accum_out=firstsq[:])
                se2 = fmp.tile([128, 16], F32, tag="se2")
                se2r = fmp.tile([128, 1], F32, tag="se2r")
                nc.scalar.activation(se2[:], s6[:, 0:16], AF.Square,
                                     accum_out=se2r[:])
                t1 = fmp.tile([128, 1], F32, tag="t1")
                nc.vector.tensor_tensor(t1[:], sqall[:], firstsq[:], op=ALU.subtract)
                t2 = fmp.tile([128, 1], F32, tag="t2")
                nc.vector.tensor_tensor(t2[:], se2r[:], t1[:], op=ALU.subtract)
                # fm = 0.5*t2 + sumfirst  -> G col 14
                nc.scalar.activation(G[:, 14:15], t2[:], AF.Identity,
                                     bias=s6[:, 16:17], scale=0.5)
                # transposes into feature-major XT chunks
                for ci, (a, b) in enumerate(K1):
                    w = b - a
                    tp = pst.tile([128, 128], F32, tag="tp")
                    nc.tensor.transpose(tp[0:w, :], G[:, a:b], ident[:])
                    nc.vector.tensor_copy(xt[ci][:, tsl], tp[0:w, :])

            # ---- MLP per 512-wide b-tile ----
            for bt in range(NBT):
                bsl = slice(bt * 512, (bt + 1) * 512)
                # layer 1
                for mi, (ma, mb) in enumerate(M1):
                    mw = mb - ma
                    ps = psm.tile([128, 512], F32, tag="ps")
                    for ci in range(4):
                        nc.tensor.matmul(ps[0:mw, :], lhsT=w1c[ci][:, ma:mb],
                                         rhs=xt[ci][:, bsl],
                                         start=(ci == 0), stop=(ci == 3))
                    if mi < 3:
                        nc.scalar.activation(h1s[mi][:, bsl], ps[:, :], AF.Relu)
                    else:
                        nc.scalar.activation(h1s3[:, bsl], ps[0:17, :], AF.Relu)
                # layer 2
                rhs2 = [h1s[0][:, bsl], h1s[1][:, bsl], h1s[2][:, bsl],
                        h1s3[:, bsl]]
                for mi, (ma, mb) in enumerate(M2):
                    mw = mb - ma
                    ps = psm.tile([128, 512], F32, tag="ps")
                    for ci in range(4):
                        nc.tensor.matmul(ps[0:mw, :], lhsT=w2c[ci][:, ma:mb],
                                         rhs=rhs2[ci],
                                         start=(ci == 0), stop=(ci == 3))
                    if mi < 3:
                        nc.scalar.activation(h2s[mi][:, bsl], ps[:, :], AF.Relu)
                    else:
                        nc.scalar.activation(h2s3[:, bsl], ps[0:16, :], AF.Relu)
                # final: out[1, 128] per 128-tile
                for tt in range(4):
                    t = bt * 4 + tt
                    tsl = slice(t * 128, (t + 1) * 128)
                    pf = psf.tile([1, 128], F32, tag="pf")
                    nc.tensor.matmul(pf[:], lhsT=wot[:, 0:1], rhs=h2s[0][:, tsl],
                                     start=True, stop=False)
                    nc.tensor.matmul(pf[:], lhsT=wot[:, 1:2], rhs=h2s[1][:, tsl],
                                     start=False, stop=False)
                    nc.tensor.matmul(pf[:], lhsT=wot[:, 2:3], rhs=h2s[2][:, tsl],
                                     start=False, stop=False)
                    nc.tensor.matmul(pf[:], lhsT=wot[0:16, 3:4], rhs=h2s3[:, tsl],
                                     start=False, stop=False)
                    nc.tensor.matmul(pf[:], lhsT=v0[:], rhs=xt[0][:, tsl],
                                     start=False, stop=True)
                    nc.vector.tensor_copy(outsb[:, tsl], pf[:])
            nc.sync.dma_start(out_d[:], outsb[:])
    nc.compile()
    return nc


class _Runner:
    def __init__(self, nc, n_cores, shared):
        install_neuronx_cc_hook()
        self.nc = nc
        self.n_cores = n_cores
        self.shared = set(shared)
        pname = nc.partition_id_tensor.name if nc.partition_id_tensor else None
        in_names, out_names, out_avals = [], [], []
        self.out_shapes = {}
        for alloc in nc.m.functions[0].allocations:
            if not isinstance(alloc, mybir.MemoryLocationSet):
                continue
            name = alloc.memorylocations[0].name
            if alloc.kind == "ExternalInput":
                if name != pname:
                    in_names.append(name)
            elif alloc.kind == "ExternalOutput":
                shape = tuple(alloc.tensor_shape)
                dtype = mybir.dt.np(alloc.dtype)
                out_names.append(name)
                out_avals.append(jax.core.ShapedArray(shape, dtype))
                self.out_shapes[name] = (shape, dtype)
        self.in_names, self.out_names = in_names, out_names
        all_in = in_names + out_names + ([pname] if pname else [])

        def _body(*args):
            ops = list(args)
            if pname:
                ops.append(partition_id_tensor())
            return tuple(_bass_exec_p.bind(
                *ops, out_avals=tuple(out_avals), in_names=tuple(all_in),
                out_names=tuple(out_names), lowering_input_output_aliases=(),
                sim_require_finite=True, sim_require_nnan=True, nc=nc))

        devs = jax.devices()[:n_cores]
        self.mesh = Mesh(np.asarray(devs), ("core",))
        in_specs = tuple(
            PartitionSpec(None) if nm in self.shared else PartitionSpec("core")
            for nm in in_names) + (PartitionSpec("core"),) * len(out_names)
        self.fn = jax.jit(
            shard_map(_body, mesh=self.mesh, in_specs=in_specs,
                      out_specs=(PartitionSpec("core"),) * len(out_names),
                      check_rep=False),
            keep_unused=True)

    def run(self, in_maps):
        args = []
        for nm in self.in_names:
            if nm in self.shared:
                a = np.ascontiguousarray(in_maps[0][nm])
                sh = NamedSharding(self.mesh, PartitionSpec(None))
            else:
                a = np.concatenate([np.asarray(m[nm]) for m in in_maps], axis=0)
                sh = NamedSharding(self.mesh, PartitionSpec("core"))
            args.append(jax.device_put(a, sh))
        for nm in self.out_names:
            shape, dtype = self.out_shapes[nm]
            z = np.zeros((self.n_cores * shape[0], *shape[1:]), dtype)
            args.append(jax.device_put(
                z, NamedSharding(self.mesh, PartitionSpec("core"))))
        outs = self.fn(*args)
        jax.block_until_ready(outs)
        self._last_args = args
        res = [dict() for _ in range(self.n_cores)]
        for i, nm in enumerate(self.out_names):
            shape, _ = self.out_shapes[nm]
            full = np.asarray(outs[i]).reshape(self.n_cores, *shape)
            for c in range(self.n_cores):
                res[c][nm] = full[c]
        return res


_CACHE = {}


def _prep_host(inputs):
    cat = np.asarray(inputs["categorical"]).astype(np.int64)
    cont = np.asarray(inputs["continuous"], dtype=np.float32)
    te = np.asarray(inputs["tables_emb"], dtype=np.float32)     # [F, V, 16]
    tf = np.asarray(inputs["tables_first"], dtype=np.float32)   # [F, V, 1]
    W1 = np.asarray(inputs["W1"], dtype=np.float32)
    b1 = np.asarray(inputs["b1"], dtype=np.float32)
    W2 = np.asarray(inputs["W2"], dtype=np.float32)
    b2 = np.asarray(inputs["b2"], dtype=np.float32)
    Wo = np.asarray(inputs["W_out"], dtype=np.float32)
    bo = np.asarray(inputs["b_out"], dtype=np.float32)
    wc = np.asarray(inputs["w_cont"], dtype=np.float32)
    bc = np.asarray(inputs["b_cont"], dtype=np.float32)

    gtab = np.concatenate(
        [te.reshape(F * V, E), tf.reshape(F * V, 1)], axis=1).astype(np.float32)
    flat = (cat + (np.arange(F, dtype=np.int64) * V)[None, :]).astype(np.int32)

    # W1 permuted to G-column order
    w1p = np.zeros((GF, H1 + 1), np.float32)
    w1p[0:CONT, 0:H1] = W1[0:CONT]
    w1p[13, 0:H1] = b1
    w1p[13, H1] = 1.0
    for f in range(F):
        w1p[HD + GW * f: HD + GW * f + E, 0:H1] = W1[CONT + E * f: CONT + E * (f + 1)]
    w2p = np.zeros((401, H2), np.float32)
    w2p[0:400] = W2
    w2p[400] = b2
    wo_pad = np.zeros(512, np.float32)
    wo_pad[:400] = Wo[1:, 0]
    wot = np.ascontiguousarray(wo_pad.reshape(4, 128).T)
    w00 = float(Wo[0, 0])
    v0 = np.zeros((128, 1), np.float32)
    v0[0:CONT, 0] = w00 * wc[:, 0]
    v0[13, 0] = float(bo[0]) + w00 * float(bc[0])
    v0[14, 0] = w00
    return gtab, flat, cont, w1p, w2p, wot, v0


def kernel(**inputs) -> np.ndarray:
    gtab, flat, cont, w1p, w2p, wot, v0 = _prep_host(inputs)
    if "nc" not in _CACHE:
        _CACHE["nc"] = _build()
        _CACHE["runner"] = _Runner(
            _CACHE["nc"], NCORES,
            shared={"gtab", "w1_d", "w2_d", "wot_d", "v0_d"})
    r = _CACHE["runner"]
    in_maps = []
    for c in range(NCORES):
        sl = slice(c * BC, (c + 1) * BC)
        in_maps.append({
            "gtab": gtab, "w1_d": w1p, "w2_d": w2p, "wot_d": wot, "v0_d": v0,
            "idx_d": np.ascontiguousarray(flat[sl]),
            "cont_d": np.ascontiguousarray(cont[sl]),
        })
    res = r.run(in_maps)
    out = np.concatenate([res[c]["out_d"].reshape(BC) for c in range(NCORES)])
    return out.reshape(B, 1).astype(np.float32)

